# revision 40
# baseline (speedup 1.0000x reference)
"""Co-attention fusion kernel for 8 TRN2 NeuronCores.

Row-parallel flash attention (per the sharding hint), S^T formulation:
- Shard rows (N=8192) of image/tabular features across 8 cores (1024 each).
- Each core projects its local K/V shards in bf16, AllGathers them in
  chunked collectives (K^T bf16, V bf16) that overlap the projections and
  the early attention compute.
- S is computed TRANSPOSED (S^T[k,q] = K^T.T @ Q^T with keys on the PSUM
  partition axis), so exp(S^T) lands in SBUF already in the layout the
  AV matmul needs as its MOVING operand.
- The AV phase keeps V blocks STATIONARY in the PE array (one weight load
  covers both 512-query streams) and produces attended^T [d, q] directly,
  which is exactly the stationary layout the output projection needs --
  no PE transposes anywhere.
- Softmax row sums come from a ones-column matmul over a vector-engine
  pairwise accumulation of exp(S^T); 1/L is computed on the [1, q] row and
  broadcast to all partitions with a rank-1 matmul, then folded into the
  AV h1 PSUM drain.
- A post-legalize pass drops LDWEIGHTS instructions whose weights AP and
  dependencies match the immediately preceding load on the PE queue
  (pl/ph pairs, AV q-halves, output-projection od-halves), roughly
  halving PE weight-load traffic.

Numerics: logits have std ~13 (range +-87). All projections and matmuls
run in bf16 (weights and activations pre-cast on host); PSUM accumulation
is fp32. Softmax uses a fixed shift M=96 instead of a row max (exp(s-96)
cannot overflow for logits < 184; actual row maxima are 44..87). The h0
AV partial is staged in bf16 (relative error <= 0.4% of the final value).
Measured end-to-end rel err ~0.011 vs the 0.02 gate.
"""

import os
import numpy as np
import ml_dtypes

import concourse.bacc as bacc
import concourse.mybir as mybir
import concourse.tile as tile
from concourse.bass_utils import run_bass_kernel_spmd

N = 8192
D = 1024
NCORES = 8
SH = N // NCORES  # rows (queries) per core
NCH = D // 128    # 8 contraction chunks
M_SHIFT = 96.0

f32 = mybir.dt.float32
f32r = mybir.dt.float32r
bf16 = mybir.dt.bfloat16

Exp = mybir.ActivationFunctionType.Exp
ADD = mybir.AluOpType.add
MULT = mybir.AluOpType.mult

# PE instruction types that do not disturb the loaded weight array
_PE_TRANSPARENT = ("InstEventSemaphore", "InstDrain", "InstNop",
                   "InstRegisterMove", "InstTPBBaseLd")


def dedup_ldweights(nc):
    """Remove PE weight loads that reload the exact weights already in the
    array: an InstLdweights whose weights AP, transpose mode, tile position
    and dependency set match the previous InstLdweights on the PE queue,
    with only non-self-loading matmuls in between. Nothing in the module
    depends on InstLdweights instructions (verified: tile_legalize moves
    only upstream edges onto them), so dropping them is dependency-safe;
    the retained earlier load carries the identical waits."""
    n_removed = 0
    for blk in nc.main_func.blocks:
        last_key = None
        keep = []
        for inst in blk.instructions:
            tn = type(inst).__name__
            if getattr(inst, "engine", None) != mybir.EngineType.PE:
                keep.append(inst)
                continue
            if tn == "InstLdweights":
                key = (
                    str(inst.ins[0]),
                    bool(getattr(inst, "is_transpose", False) or False),
                    getattr(inst, "tile_position", None),
                    getattr(inst, "tile_size", None),
                    getattr(inst, "perf_mode", None),
                    tuple(sorted(inst.sync_dependency_names())),
                )
                if key == last_key:
                    n_removed += 1
                    continue  # drop: identical weights already loaded
                last_key = key
                keep.append(inst)
            elif tn == "InstMatmult":
                if getattr(inst, "ldweights", None) is not False:
                    # self-loading (f32/f32r fused path) clobbers the array
                    last_key = None
                keep.append(inst)
            elif tn in _PE_TRANSPARENT:
                keep.append(inst)
            else:
                last_key = None
                keep.append(inst)
        if len(keep) != len(blk.instructions):
            blk.instructions[:] = keep
    return n_removed


def build_nc():
    nc = bacc.Bacc(trn_type="TRN2", num_devices=NCORES)

    # ---- parameters ----
    xTi = nc.declare_dram_parameter("xTi", [D, SH], bf16, isOutput=False)
    xTt = nc.declare_dram_parameter("xTt", [D, SH], bf16, isOutput=False)
    Ws = {
        name: nc.declare_dram_parameter(name, [D, D], bf16, isOutput=False)
        for name in ["Wqi", "Wkt", "Wvt", "Wqt", "Wki", "Wvi"]
    }
    Wo16 = nc.declare_dram_parameter("Wo16", [2 * D, 2 * D], bf16, isOutput=False)
    Bs = {
        name: nc.declare_dram_parameter(name, [1, D], f32, isOutput=False)
        for name in ["bqi", "bkt", "bvt", "bqt", "bki", "bvi"]
    }
    bo32 = nc.declare_dram_parameter("bo32", [1, 2 * D], f32, isOutput=False)
    ones32 = nc.declare_dram_parameter("ones32", [1, 128], f32, isOutput=False)
    onescol = nc.declare_dram_parameter("onescol", [128, 1], f32, isOutput=False)
    out = nc.declare_dram_parameter("out", [SH, 2 * D], f32, isOutput=True)

    # ---- internal DRAM ----
    # Per-branch, per-key-half AllGather bounces. K^T is stored pre-tiled as
    # [c-chunk, 128 d, 256 local keys] bf16; V natural [512 local keys, D].
    bk = [[[nc.dram_tensor(f"bk{b}{h}{j}", [NCH, 128, 256], bf16)
            for j in range(2)] for h in range(2)] for b in range(2)]
    gk = [[[nc.dram_tensor(f"gk{b}{h}{j}", [NCORES * NCH, 128, 256], bf16,
                           addr_space="Shared") for j in range(2)]
           for h in range(2)] for b in range(2)]
    bv = [[nc.dram_tensor(f"bv{b}{h}", [512, D], bf16) for h in range(2)]
          for b in range(2)]
    gv = [[nc.dram_tensor(f"gv{b}{h}", [NCORES * 512, D], bf16,
                          addr_space="Shared") for h in range(2)]
          for b in range(2)]
    qT1_dram = nc.dram_tensor("qT1", [D, SH], bf16)

    rg = [list(range(NCORES))]

    def ch(handle2d):
        """DRAM [R, C] -> [128, R/128, C] AP (partition=row%128, chunked)."""
        return handle2d[:, :].rearrange("(c p) x -> p c x", p=128)

    with tile.TileContext(nc) as tc:
        with (
            tc.tile_pool(name="po", bufs=1) as po,       # small consts, persistent
            tc.tile_pool(name="poq", bufs=1) as poq,     # q^T slot (reused per branch)
        ):
            onescol_sb = po.tile([128, 1], f32r, tag="onescol")
            ones_row = po.tile([1, 128], f32r, tag="ones_row")
            negm = po.tile([128, 1], f32, tag="negm")
            lsum_row = po.tile([1, SH], f32r, tag="lsum_row")
            linv_row = po.tile([1, SH], f32r, tag="linv_row")
            linv_bc = po.tile([128, SH], f32, tag="linv_bc")
            bo_bc = po.tile([128, 2 * D], f32, tag="bo_bc")

            nc.vector.memset(negm[:], -M_SHIFT)

            # ============ stage 1: projections + chunked AllGathers ============
            with (
                tc.tile_pool(name="s1", bufs=1) as s1,
                tc.tile_pool(name="s1w", bufs=1) as s1w,
                tc.tile_pool(name="s1s", bufs=4) as s1s,
                tc.tile_pool(name="ps1", bufs=4, space="PSUM") as ps1,
            ):
                # Ring discipline: each HWDGE queue serves ALL outstanding
                # descriptors concurrently (packets striped over 16 engines),
                # so delivery order can only be enforced by holding later
                # transfers back with dependencies. Weights go SCALAR-ONLY in
                # dependency-gated waves (each released by the first drain of
                # an earlier projection); x, biases and all bounce writes go
                # on SYNC so a waiting wave never blocks them.
                rings = (nc.sync, nc.scalar)

                xtt = s1.tile([128, NCH, SH], bf16, tag="xtt")
                nc.sync.dma_start(out=xtt[:], in_=ch(xTt))
                wt_ = {}
                wt_["Wkt"] = s1w.tile([128, NCH, D], bf16, tag="wWkt", name="wWkt")
                wap0 = ch(Ws["Wkt"])
                nc.scalar.dma_start(out=wt_["Wkt"][:, :, 0:512],
                                    in_=wap0[:, :, 0:512])
                # per-out-channel biases for q/k projections ([d_out%128, chunk])
                bcol = {}
                bcol["bkt"] = s1.tile([128, NCH], f32, tag="bkt", name="bcol_bkt")
                nc.sync.dma_start(
                    out=bcol["bkt"][:],
                    in_=Bs["bkt"][0, :].rearrange("(c p) -> p c", p=128),
                )
                nc.scalar.dma_start(out=wt_["Wkt"][:, :, 512:1024],
                                    in_=wap0[:, :, 512:1024])
                xti = s1.tile([128, NCH, SH], bf16, tag="xti")
                nc.sync.dma_start(out=xti[:], in_=ch(xTi))
                for bn in ("bqi", "bki", "bqt"):
                    bcol[bn] = s1.tile([128, NCH], f32, tag=bn, name="bcol_" + bn)
                    nc.sync.dma_start(
                        out=bcol[bn][:],
                        in_=Bs[bn][0, :].rearrange("(c p) -> p c", p=128),
                    )
                for wn in ("Wqi", "Wvt", "Wki", "Wqt", "Wvi"):
                    wt_[wn] = s1w.tile([128, NCH, D], bf16, tag="w" + wn,
                                       name="w" + wn)

                def load_w_wave(wn, dep):
                    wap = ch(Ws[wn])
                    di = None
                    for half in range(2):
                        di = nc.scalar.dma_start(
                            out=wt_[wn][:, :, half * 512:(half + 1) * 512],
                            in_=wap[:, :, half * 512:(half + 1) * 512],
                        )
                        if dep is not None:
                            tile.add_dep_helper(
                                di.ins, dep.ins,
                                reason=f"stage {wn} load wave release")
                    return di
                nc.sync.dma_start(out=ones_row[:], in_=ones32[:, :].bitcast(f32r))
                nc.sync.dma_start(out=onescol_sb[:], in_=onescol[:, :].bitcast(f32r))
                brow = {}
                for bn in ("bvt", "bvi"):
                    brow[bn] = s1.tile([1, D], f32r, tag="br" + bn, name="br" + bn)
                    nc.sync.dma_start(out=brow[bn][:], in_=Bs[bn][:, :].bitcast(f32r))
                bo_row = s1.tile([1, 2 * D], f32r, tag="bo_row")
                nc.sync.dma_start(out=bo_row[:], in_=bo32[:, :].bitcast(f32r))
                bv_bc = {}

                def make_bv_bc(bn):
                    # broadcast v-bias to all 128 partitions via rank-1 matmul
                    bv_bc[bn] = s1.tile([128, D], f32, tag="bc" + bn, name="bc" + bn)
                    for j in range(2):
                        ps = ps1.tile([128, 512], f32, tag="pp")
                        nc.tensor.matmul(
                            ps[:], ones_row[:, :],
                            brow[bn][:, j * 512:(j + 1) * 512],
                            start=True, stop=True,
                        )
                        nc.vector.tensor_copy(bv_bc[bn][:, j * 512:(j + 1) * 512], ps[:])

                def proj_T(wname, bname, xt, dst, w=None):
                    """K^T/Q^T projection: out[d_out, rows].

                    dst: ("dram2", (t_half0, t_half1)) pre-tiled [NCH,128,256],
                         ("dramq", tensor [D, SH]), or ("sbuf", tile [128,NCH,SH]).
                    The pl/ph pair shares one PE weight load (dedup pass).
                    Returns its first drain instruction (wave release point).
                    """
                    w = wt_[wname] if w is None else w
                    first_drain = None
                    kind, tgt = dst
                    for od in range(NCH):
                        pss = [ps1.tile([128, 512], f32, tag="pp", name=f"pp{_i}")
                               for _i in range(2)]
                        for c in range(NCH):
                            lhs = w[:, c, od * 128:(od + 1) * 128]
                            for rt in range(2):
                                nc.tensor.matmul(
                                    pss[rt][:], lhs,
                                    xt[:, c, rt * 512:(rt + 1) * 512],
                                    start=(c == 0), stop=(c == NCH - 1),
                                )
                        for rt in range(2):
                            if kind == "sbuf":
                                di = nc.vector.tensor_scalar_add(
                                    tgt[:, od, rt * 512:(rt + 1) * 512],
                                    pss[rt][:], bcol[bname][:, od:od + 1],
                                )
                            elif kind == "dram2":
                                stg = s1s.tile([128, 512], bf16, tag="stgk",
                                               name="stgk")
                                di = nc.vector.tensor_scalar_add(
                                    stg[:], pss[rt][:], bcol[bname][:, od:od + 1]
                                )
                                for j in range(2):
                                    nc.sync.dma_start(
                                        out=tgt[rt][j][od, :, :],
                                        in_=stg[:, j * 256:(j + 1) * 256],
                                    )
                            else:
                                stg = s1s.tile([128, 512], bf16, tag="stgk",
                                               name="stgq")
                                di = nc.vector.tensor_scalar_add(
                                    stg[:], pss[rt][:], bcol[bname][:, od:od + 1]
                                )
                                nc.sync.dma_start(
                                    out=tgt[od * 128:(od + 1) * 128,
                                            rt * 512:(rt + 1) * 512],
                                    in_=stg[:],
                                )
                            if first_drain is None:
                                first_drain = di
                    return first_drain

                def proj_V(wname, bname, xt, tgts):
                    """v projection, natural [rows, d_out] -> bf16 half bounces."""
                    w = wt_[wname]
                    first_drain = None
                    for rt in range(NCH):
                        pss = [ps1.tile([128, 512], f32, tag="pp", name=f"pp{_i}")
                               for _i in range(2)]
                        for c in range(NCH):
                            lhs = xt[:, c, rt * 128:(rt + 1) * 128]
                            for ot in range(2):
                                nc.tensor.matmul(
                                    pss[ot][:], lhs,
                                    w[:, c, ot * 512:(ot + 1) * 512],
                                    start=(c == 0), stop=(c == NCH - 1),
                                )
                        for ot in range(2):
                            stg = s1s.tile([128, 512], bf16, tag="vstg")
                            di = nc.vector.scalar_tensor_tensor(
                                stg[:], pss[ot][:], 0.0,
                                bv_bc[bname][:, ot * 512:(ot + 1) * 512],
                                op0=ADD, op1=ADD,
                            )
                            if first_drain is None:
                                first_drain = di
                            nc.sync.dma_start(
                                out=tgts[rt // 4][(rt % 4) * 128:(rt % 4 + 1) * 128,
                                                  ot * 512:(ot + 1) * 512],
                                in_=stg[:],
                            )
                    return first_drain

                def ag(src_t, dst_t):
                    return nc.gpsimd.collective_compute(
                        "AllGather", mybir.AluOpType.bypass,
                        replica_groups=rg,
                        ins=[src_t.ap().opt()], outs=[dst_t.ap().opt()],
                    )

                qt0 = poq.tile([128, NCH, SH], bf16, tag="qt", name="qt0")

                # Projection order: Wkt, Wqi, Wvt, Wki, Wqt, Wvi. Each weight
                # wave is released by the previous projection's first drain so
                # deliveries track consumption. The gather chain (which
                # crushes ring bandwidth while running) is gated behind the
                # LAST weight wave -- every gather deadline has 100us+ of
                # slack, the weight deadlines don't.
                d_kt = proj_T("Wkt", "bkt", xtt, ("dram2", bk[0]),
                              w=wt_["Wkt"])
                load_w_wave("Wqi", d_kt)
                make_bv_bc("bvt")
                make_bv_bc("bvi")
                # broadcast output bias now, off the stage-3 critical path
                for j in range(4):
                    ps = ps1.tile([128, 512], f32, tag="pp")
                    nc.tensor.matmul(
                        ps[:], ones_row[:, :], bo_row[:, j * 512:(j + 1) * 512],
                        start=True, stop=True,
                    )
                    nc.vector.tensor_copy(bo_bc[:, j * 512:(j + 1) * 512], ps[:])
                d_qi = proj_T("Wqi", "bqi", xti, ("sbuf", qt0))
                load_w_wave("Wvt", d_qi)
                d_vt = proj_V("Wvt", "bvt", xtt, bv[0])
                load_w_wave("Wki", d_vt)
                d_ki = proj_T("Wki", "bki", xti, ("dram2", bk[1]))
                load_w_wave("Wqt", d_ki)
                d_qt = proj_T("Wqt", "bqt", xtt, ("dramq", qT1_dram))
                wvi_dma = load_w_wave("Wvi", d_qt)
                proj_V("Wvi", "bvi", xti, bv[1])
                g1 = ag(bk[0][0][0], gk[0][0][0])
                tile.add_dep_helper(g1.ins, wvi_dma.ins,
                                    reason="gathers after weight front-load")
                ag(bk[0][0][1], gk[0][0][1])
                ag(bv[0][0], gv[0][0])
                ag(bk[0][1][0], gk[0][1][0])
                ag(bk[0][1][1], gk[0][1][1])
                ag(bv[0][1], gv[0][1])
                ag(bk[1][0][0], gk[1][0][0])
                ag(bk[1][0][1], gk[1][0][1])
                ag(bv[1][0], gv[1][0])
                ag(bk[1][1][0], gk[1][1][0])
                ag(bk[1][1][1], gk[1][1][1])
                ag(bv[1][1], gv[1][1])

            # ============ stage 2: attention (flash, S^T form) ============
            # fused^T accumulator [fused_dim, q] lives from here through the
            # output projection.
            pf = tc.alloc_tile_pool(name="pf", bufs=1)
            fusedbf = pf.tile([128, 2 * NCH, SH], bf16, tag="fusedbf",
                              name="fusedbf")
            wo_pre = pf.tile([128, 2 * NCH, 512], bf16, tag="wo_pre",
                             name="wo_pre")

            with (
                tc.tile_pool(name="sA", bufs=1) as sA,
                tc.tile_pool(name="sK", bufs=6) as sK,
                tc.tile_pool(name="sV", bufs=4) as sV,
                tc.tile_pool(name="sT", bufs=2) as sT,
            ):
                A = sA.tile([128, 32, SH], bf16, tag="A")

                # deep kt/vt buffering + cross-phase prefetch ride out the
                # ~35us windows where a running AllGather starves the rings
                pre_kt = {}
                pre_vt = {}

                def _kt_issue(b, h, q2, r, split=False):
                    kt = sK.tile([128, NCH, 256], bf16, tag="kt", name="kt")
                    src = gk[b][h][q2][r * NCH:(r + 1) * NCH, :, :] \
                        .rearrange("c p k -> p c k")
                    if split:
                        # latency-critical: halves on both rings
                        nc.sync.dma_start(out=kt[:, 0:4, :], in_=src[:, 0:4, :])
                        nc.scalar.dma_start(out=kt[:, 4:8, :], in_=src[:, 4:8, :])
                    else:
                        kdma = nc.sync if r % 2 == 0 else nc.scalar
                        kdma.dma_start(out=kt[:], in_=src)
                    return kt

                def kt_load(b, h, q2, r):
                    t = pre_kt.pop((b, h, q2, r), None)
                    return t if t is not None else _kt_issue(b, h, q2, r)

                def _vt_issue(b, h, dh, g, split=False):
                    vt = sV.tile([128, 4, 512], bf16, tag="vt")
                    src = gv[b][h][g * 512:(g + 1) * 512,
                                   dh * 512:(dh + 1) * 512] \
                        .rearrange("(j p) d -> p j d", p=128)
                    if split:
                        nc.sync.dma_start(out=vt[:, 0:2, :], in_=src[:, 0:2, :])
                        nc.scalar.dma_start(out=vt[:, 2:4, :], in_=src[:, 2:4, :])
                    else:
                        vdma = nc.sync if g % 2 == 0 else nc.scalar
                        vdma.dma_start(out=vt[:], in_=src)
                    return vt

                def vt_load(b, h, dh, g):
                    t = pre_vt.pop((b, h, dh, g), None)
                    return t if t is not None else _vt_issue(b, h, dh, g)

                # prefetch the first kt tiles of branch 0 before the S loop
                for r0 in range(2):
                    pre_kt[(0, 0, 0, r0)] = _kt_issue(0, 0, 0, r0, split=True)

                qt1 = [None]
                for b in range(2):
                    if b == 0:
                        qt = qt0
                    else:
                        qt = qt1[0]
                    fofs8 = NCH if b == 0 else 0  # b0 -> attended_tabular

                    acc = sT.tile([128, SH], f32r, tag="acc", name="acc",
                                  bufs=1)
                    for h in range(2):
                        # ---- S phase: A[k,q] = exp(K^T.T @ Q^T - M) ----
                        with (
                            tc.tile_pool(name="psS", bufs=4, space="PSUM") as psS,
                        ):
                            for q2 in range(2):
                                for r in range(NCORES):
                                    kt = kt_load(b, h, q2, r)
                                    if q2 == 0 and r == 4:
                                        # prefetch q2=1's first kt tiles (their
                                        # gather is typically done by now)
                                        for r0 in range(2):
                                            pre_kt[(b, h, 1, r0)] = _kt_issue(
                                                b, h, 1, r0)
                                    if q2 == 1 and r == 4:
                                        # prefetch first AV vt tiles of this
                                        # (b, h) while S still computes
                                        for g0 in range(2):
                                            pre_vt[(b, h, 0, g0)] = _vt_issue(
                                                b, h, 0, g0, split=True)
                                    if b == 1 and h == 1 and q2 == 0 and r == 6:
                                        # prefetch first Wo od-chunk for the
                                        # output projection (4 ring-split
                                        # chunks so it lands promptly)
                                        wo_src = Wo16[:, 0:512].rearrange(
                                            "(c p) o -> p c o", p=128)
                                        for i in range(4):
                                            rings[i % 2].dma_start(
                                                out=wo_pre[:, i * 4:(i + 1) * 4, :],
                                                in_=wo_src[:, i * 4:(i + 1) * 4, :],
                                            )
                                    for jj in range(2):
                                        idx = q2 * 16 + r * 2 + jj
                                        pl = psS.tile([128, 512], f32, tag="s",
                                                      name="pl")
                                        ph = psS.tile([128, 512], f32, tag="s",
                                                      name="ph")
                                        for c in range(NCH):
                                            lhs = kt[:, c, jj * 128:(jj + 1) * 128]
                                            nc.tensor.matmul(
                                                pl[:], lhs, qt[:, c, 0:512],
                                                start=(c == 0), stop=(c == NCH - 1),
                                            )
                                            nc.tensor.matmul(
                                                ph[:], lhs, qt[:, c, 512:1024],
                                                start=(c == 0), stop=(c == NCH - 1),
                                            )
                                        nc.scalar.activation(
                                            A[:, idx, 0:512], pl[:], Exp,
                                            bias=negm[:, 0:1], scale=1.0,
                                        )
                                        nc.scalar.activation(
                                            A[:, idx, 512:1024], ph[:], Exp,
                                            bias=negm[:, 0:1], scale=1.0,
                                        )
                                        # fold exp'd blocks pairwise into the
                                        # branch row-sum accumulator
                                        if idx % 2 == 1:
                                            t2 = sT.tile([128, SH], f32r, tag="t2",
                                                         name="t2", bufs=2)
                                            nc.vector.scalar_tensor_tensor(
                                                t2[:], A[:, idx - 1, :], 0.0,
                                                A[:, idx, :], op0=ADD, op1=ADD,
                                            )
                                            if h == 0 and idx == 1:
                                                nc.vector.tensor_copy(acc[:], t2[:])
                                            else:
                                                nc.vector.scalar_tensor_tensor(
                                                    acc[:], t2[:], 0.0, acc[:],
                                                    op0=ADD, op1=ADD,
                                                )
                            if h == 1:
                                # partition-reduce acc via a ones-matmul, then
                                # 1/L on the [1,q] row, broadcast to 128
                                # partitions with a rank-1 matmul
                                for j in range(2):
                                    lsT = psS.tile([1, 512], f32, tag="lsT",
                                                   name="lsT", bufs=1)
                                    nc.tensor.matmul(
                                        lsT[:], onescol_sb[:, :],
                                        acc[:, j * 512:(j + 1) * 512],
                                        start=True, stop=True,
                                    )
                                    nc.vector.tensor_copy(
                                        lsum_row[0:1, j * 512:(j + 1) * 512],
                                        lsT[:],
                                    )
                                with nc.allow_low_precision(
                                        reason="f32r is f32 bits"):
                                    nc.vector.reciprocal(linv_row[:],
                                                         lsum_row[:])
                                for j in range(2):
                                    bcp = psS.tile([128, 512], f32, tag="bc",
                                                   name="bcp", bufs=2)
                                    nc.tensor.matmul(
                                        bcp[:], ones_row[:, :],
                                        linv_row[0:1, j * 512:(j + 1) * 512],
                                        start=True, stop=True,
                                    )
                                    nc.vector.tensor_copy(
                                        linv_bc[:, j * 512:(j + 1) * 512], bcp[:]
                                    )

                        # ---- AV phase: attended^T += V^T-blocks @ A ----
                        # V block [k128, dv128] is the stationary operand; one
                        # weight load streams both 512-query halves of A.
                        with tc.tile_pool(name="psA", bufs=8, space="PSUM") as psA:
                            for dh in range(2):
                                avp = [
                                    [psA.tile([128, 512], f32, tag="av",
                                              name=f"av{dvb}{qh}", bufs=8)
                                     for qh in range(2)]
                                    for dvb in range(4)
                                ]
                                for g in range(NCORES):
                                    vt = vt_load(b, h, dh, g)
                                    if (b == 0 and h == 1 and dh == 0
                                            and g == 3):
                                        # prefetch branch-1 q^T (WAR on qt0
                                        # resolved: last S matmul has read it)
                                        qt1[0] = poq.tile([128, NCH, SH], bf16,
                                                          tag="qt", name="qt1")
                                        qsrc = ch(qT1_dram)
                                        for i in range(4):
                                            rings[i % 2].dma_start(
                                                out=qt1[0][:, i * 2:(i + 1) * 2, :],
                                                in_=qsrc[:, i * 2:(i + 1) * 2, :],
                                            )
                                    if dh == 1 and g == 5 and (b, h) != (1, 1):
                                        # prefetch next S phase's first kt
                                        nb, nh = (b, 1) if h == 0 else (1 - b, 0)
                                        for r0 in range(2):
                                            pre_kt[(nb, nh, 0, r0)] = _kt_issue(
                                                nb, nh, 0, r0, split=True)
                                    for j in range(4):
                                        idx = (j // 2) * 16 + g * 2 + (j % 2)
                                        kb = g * 4 + j
                                        for dvb in range(4):
                                            lhs = vt[:, j, dvb * 128:(dvb + 1) * 128]
                                            for qh in range(2):
                                                nc.tensor.matmul(
                                                    avp[dvb][qh][:], lhs,
                                                    A[:, idx,
                                                      qh * 512:(qh + 1) * 512],
                                                    start=(kb == 0),
                                                    stop=(kb == 31),
                                                )
                                for dvb in range(4):
                                    fch = fofs8 + dh * 4 + dvb
                                    for qh in range(2):
                                        sl = fusedbf[:, fch,
                                                     qh * 512:(qh + 1) * 512]
                                        if h == 0:
                                            nc.vector.tensor_copy(
                                                sl, avp[dvb][qh][:]
                                            )
                                        else:
                                            tmp = sT.tile([128, 512], f32,
                                                          tag="tmp")
                                            nc.vector.scalar_tensor_tensor(
                                                tmp[:], avp[dvb][qh][:], 0.0,
                                                sl, op0=ADD, op1=ADD,
                                            )
                                            nc.vector.scalar_tensor_tensor(
                                                sl, tmp[:], 0.0,
                                                linv_bc[:,
                                                        qh * 512:(qh + 1) * 512],
                                                op0=ADD, op1=MULT,
                                            )

            # ============ stage 3: output projection ============
            # fusedbf already holds fused^T [fused_dim, q]; contract over the
            # 16 f-chunks in four od-512 rounds. Round 0's Wo chunk was
            # prefetched during branch 1; each later chunk streams during the
            # previous round's matmuls.
            with (
                tc.tile_pool(name="sW2", bufs=2) as sW2,
                tc.tile_pool(name="sO", bufs=4) as sO,
                tc.tile_pool(name="psO", bufs=4, space="PSUM") as psO,
            ):
                wos = [wo_pre]
                for odr in range(1, 4):
                    wot = sW2.tile([128, 2 * NCH, 512], bf16, tag="wo",
                                   name=f"wo{odr}")
                    # odr 2-3 on scalar: the out writes stream on sync and
                    # must not queue behind a WAR-stalled wo chunk
                    ring = nc.sync if odr == 1 else nc.scalar
                    ring.dma_start(
                        out=wot[:],
                        in_=Wo16[:, odr * 512:(odr + 1) * 512]
                        .rearrange("(c p) o -> p c o", p=128),
                    )
                    wos.append(wot)

                for odr in range(4):
                    wo = wos[odr]
                    for q8 in range(NCH):
                        ps = psO.tile([128, 512], f32, tag="o", name="po")
                        for f in range(2 * NCH):
                            nc.tensor.matmul(
                                ps[:], fusedbf[:, f, q8 * 128:(q8 + 1) * 128],
                                wo[:, f, :],
                                start=(f == 0), stop=(f == 2 * NCH - 1),
                            )
                        ost = sO.tile([128, 512], f32, tag="ost")
                        nc.vector.scalar_tensor_tensor(
                            ost[:], ps[:], 0.0,
                            bo_bc[:, odr * 512:(odr + 1) * 512],
                            op0=ADD, op1=ADD,
                        )
                        nc.sync.dma_start(
                            out=out[q8 * 128:(q8 + 1) * 128,
                                    odr * 512:(odr + 1) * 512],
                            in_=ost[:],
                        )

            pf.release()

    n = dedup_ldweights(nc)
    nc.compile()
    nc._n_ldw_removed = n
    return nc


_CACHE: dict = {}


def kernel(
    image_features, tabular_features,
    Wqi, bqi, Wkt, bkt, Wvt, bvt,
    Wqt, bqt, Wki, bki, Wvi, bvi,
    Wo, bo,
) -> np.ndarray:
    if "nc" not in _CACHE:
        _CACHE["nc"] = build_nc()
    nc = _CACHE["nc"]

    bfc = lambda a: np.asarray(a, np.float32).astype(ml_dtypes.bfloat16)
    img = np.asarray(image_features, np.float32)
    tab = np.asarray(tabular_features, np.float32)
    shared = {
        "Wqi": bfc(Wqi), "Wkt": bfc(Wkt),
        "Wvt": bfc(Wvt), "Wqt": bfc(Wqt),
        "Wki": bfc(Wki), "Wvi": bfc(Wvi),
        "Wo16": np.asarray(Wo).astype(ml_dtypes.bfloat16),
        "bqi": np.asarray(bqi, np.float32).reshape(1, D),
        "bkt": np.asarray(bkt, np.float32).reshape(1, D),
        "bvt": np.asarray(bvt, np.float32).reshape(1, D),
        "bqt": np.asarray(bqt, np.float32).reshape(1, D),
        "bki": np.asarray(bki, np.float32).reshape(1, D),
        "bvi": np.asarray(bvi, np.float32).reshape(1, D),
        "bo32": np.asarray(bo, np.float32).reshape(1, 2 * D),
        "ones32": np.ones((1, 128), np.float32),
        "onescol": np.ones((128, 1), np.float32),
    }
    in_maps = []
    for c in range(NCORES):
        m = dict(shared)
        m["xTi"] = np.ascontiguousarray(img[c * SH:(c + 1) * SH, :].T).astype(
            ml_dtypes.bfloat16)
        m["xTt"] = np.ascontiguousarray(tab[c * SH:(c + 1) * SH, :].T).astype(
            ml_dtypes.bfloat16)
        in_maps.append(m)

    trace = bool(int(os.environ.get("KERNEL_TRACE", "0")))
    res = run_bass_kernel_spmd(
        nc, in_maps, core_ids=list(range(NCORES)), trace=trace
    )
    _CACHE["last_result"] = res
    return np.concatenate([res.results[c]["out"] for c in range(NCORES)], axis=0)
